# revision 10
# baseline (speedup 1.0000x reference)
"""Masked per-sample MSE loss (duration-predictor loss) on 8 Trainium2 cores.

Math (per the reference):
    mask[i, j]  = j < token_lengths[i]
    diff        = where(mask, pred - log(alignment), 0)
    out         = mean_i( sum_j diff[i,j]^2 / token_lengths[i] )

Sharding: pure data parallel over the batch dim. Each of the 8 cores gets
512 rows ([512, 2048] f32 pred + alignment). On-device, each core reduces its
rows to 512 masked sum-of-squares (as a [128, 4] tile); the tiny per-row
divide by length and the global mean run on the host in float64.

Per-core device pipeline, per [128, 2048] row-tile:
    ACT : la = Ln(align)
    DVE : d  = pred - la
    DVE : dm = (iota <f32 is_lt> len) * d        (fused scalar_tensor_tensor)
    ACT : sq = Square(dm), accum_out -> rowsum[128, 1]
DMA-bound by the 8 MiB/core of input traffic.

Written in raw Bass (explicit semaphores): the walrus build in this
environment rejects compute instructions carrying more than one sync-wait,
which the Tile scheduler emits freely — so waits are issued as standalone
wait_ge instructions instead.
"""

import numpy as np

import concourse.bass as bass
from concourse import mybir
from concourse.bass_utils import run_bass_kernel_spmd

B, T = 4096, 2048
N_CORES = 8
RPC = B // N_CORES  # rows per core = 512
P = 128             # SBUF partitions
N_TILES = RPC // P  # row-tiles per core = 4

_CACHE: dict = {}

F32 = mybir.dt.float32


def _build_module() -> bass.Bass:
    nc = bass.Bass("TRN2")

    pred_d = nc.dram_tensor("pred", [RPC, T], F32, kind="ExternalInput")
    align_d = nc.dram_tensor("align", [RPC, T], F32, kind="ExternalInput")
    lens_d = nc.dram_tensor("lens", [P, N_TILES], F32, kind="ExternalInput")
    out_d = nc.dram_tensor("rowsums", [P, N_TILES], F32, kind="ExternalOutput")

    from contextlib import ExitStack

    with ExitStack() as ctx:
        pred_sb = ctx.enter_context(nc.sbuf_tensor("pred_sb", [P, N_TILES, T], F32))
        align_sb = ctx.enter_context(nc.sbuf_tensor("align_sb", [P, N_TILES, T], F32))
        la_sb = ctx.enter_context(nc.sbuf_tensor("la_sb", [P, N_TILES, T], F32))
        d_sb = ctx.enter_context(nc.sbuf_tensor("d_sb", [P, 2, T], F32))
        dm_sb = ctx.enter_context(nc.sbuf_tensor("dm_sb", [P, 2, T], F32))
        sq_sb = ctx.enter_context(nc.sbuf_tensor("sq_sb", [P, T], F32))
        iota_i = ctx.enter_context(nc.sbuf_tensor("iota_i", [P, T], mybir.dt.int32))
        iota_f = ctx.enter_context(nc.sbuf_tensor("iota_f", [P, T], F32))
        lens_sb = ctx.enter_context(nc.sbuf_tensor("lens_sb", [P, N_TILES], F32))
        rs_sb = ctx.enter_context(nc.sbuf_tensor("rs_sb", [P, N_TILES], F32))
        s_pred = [ctx.enter_context(nc.semaphore(f"s_pred{t}")) for t in range(N_TILES)]
        s_align = [ctx.enter_context(nc.semaphore(f"s_align{t}")) for t in range(N_TILES)]
        s_lens = ctx.enter_context(nc.semaphore("s_lens"))
        s_out = ctx.enter_context(nc.semaphore("s_out"))
        s_iota = ctx.enter_context(nc.semaphore("s_iota"))
        s_iotaf = ctx.enter_context(nc.semaphore("s_iotaf"))
        s_la = ctx.enter_context(nc.semaphore("s_la"))
        s_d = ctx.enter_context(nc.semaphore("s_d"))
        s_dm = ctx.enter_context(nc.semaphore("s_dm"))
        s_sq = ctx.enter_context(nc.semaphore("s_sq"))
        block = ctx.enter_context(nc.Block())

        @block.sync
        def _(sync):
            sync.dma_start(lens_sb[:, :], lens_d[:, :]).then_inc(s_lens, 16)
            for t in range(N_TILES):
                sync.dma_start(
                    pred_sb[:, t, :], pred_d[t * P:(t + 1) * P, :]
                ).then_inc(s_pred[t], 16)
                sync.dma_start(
                    align_sb[:, t, :], align_d[t * P:(t + 1) * P, :]
                ).then_inc(s_align[t], 16)
            sync.wait_ge(s_sq, N_TILES)
            sync.dma_start(out_d[:, :], rs_sb[:, :]).then_inc(s_out, 16)
            sync.wait_ge(s_out, 16)

        @block.gpsimd
        def _(gpsimd):
            gpsimd.iota(
                iota_i[:, :], pattern=[[1, T]], base=0, channel_multiplier=0
            ).then_inc(s_iota, 1)

        @block.vector
        def _(vector):
            vector.wait_ge(s_iota, 1)
            vector.tensor_copy(iota_f[:, :], iota_i[:, :]).then_inc(s_iotaf, 1)
            vector.wait_ge(s_lens, 16)
            vector.wait_ge(s_iotaf, 1)  # same-engine RAW: iota_f visible
            for t in range(N_TILES):
                vector.wait_ge(s_pred[t], 16)
                vector.wait_ge(s_la, t + 1)
                if t >= 2:
                    # d buffer reuse: dm(t-2) must have read d_sb[t%2]
                    vector.wait_ge(s_dm, t - 1)
                vector.tensor_sub(
                    d_sb[:, t % 2, :], pred_sb[:, t, :], la_sb[:, t, :]
                ).then_inc(s_d, 1)
                if t >= 2:
                    # dm buffer reuse: Square(t-2) must have read dm_sb[t%2]
                    vector.wait_ge(s_sq, t - 1)
                vector.wait_ge(s_d, t + 1)  # same-engine RAW: d visible
                vector.scalar_tensor_tensor(
                    out=dm_sb[:, t % 2, :],
                    in0=iota_f[:, :],
                    scalar=lens_sb[:, t:t + 1],
                    in1=d_sb[:, t % 2, :],
                    op0=mybir.AluOpType.is_lt,
                    op1=mybir.AluOpType.mult,
                ).then_inc(s_dm, 1)

        @block.scalar
        def _(scalar):
            for t in range(N_TILES):
                scalar.wait_ge(s_align[t], 16)
                scalar.activation(
                    la_sb[:, t, :], align_sb[:, t, :],
                    mybir.ActivationFunctionType.Ln,
                ).then_inc(s_la, 1)
            for t in range(N_TILES):
                scalar.wait_ge(s_dm, t + 1)
                if t >= 1:
                    # same-engine WAW on sq_sb scratch
                    scalar.wait_ge(s_sq, t)
                scalar.activation(
                    sq_sb[:, :], dm_sb[:, t % 2, :],
                    mybir.ActivationFunctionType.Square,
                    accum_out=rs_sb[:, t:t + 1],
                ).then_inc(s_sq, 1)

    return nc


def _get_module() -> bass.Bass:
    if "nc" not in _CACHE:
        _CACHE["nc"] = _build_module()
    return _CACHE["nc"]


def _make_in_maps(pred, align, lens_f32):
    in_maps = []
    for c in range(N_CORES):
        sl = slice(c * RPC, (c + 1) * RPC)
        # lens laid out so column t holds the lengths of row-tile t
        lens_c = np.ascontiguousarray(lens_f32[sl].reshape(N_TILES, P).T)
        in_maps.append({
            "pred": np.ascontiguousarray(pred[sl]),
            "align": np.ascontiguousarray(align[sl]),
            "lens": lens_c,
        })
    return in_maps


def _combine(results, lens) -> np.ndarray:
    total = 0.0
    for c in range(N_CORES):
        rs = np.asarray(results[c]["rowsums"], dtype=np.float64)  # [128, N_TILES]
        per_row = rs.T.reshape(RPC)  # row r of this core's shard
        lc = lens[c * RPC:(c + 1) * RPC].astype(np.float64)
        total += np.sum(per_row / lc)
    return np.array(total / B, dtype=np.float32)


def run(inputs, trace: bool = False):
    """Returns (output, BassKernelResults). trace=True also profiles core 0."""
    pred = np.asarray(inputs["pred"], dtype=np.float32)
    align = np.asarray(inputs["alignment"], dtype=np.float32)
    lens = np.asarray(inputs["token_lengths"])
    lens_f32 = lens.astype(np.float32)

    nc = _get_module()
    in_maps = _make_in_maps(pred, align, lens_f32)
    res = run_bass_kernel_spmd(nc, in_maps, core_ids=list(range(N_CORES)), trace=trace)
    return _combine(res.results, lens), res


def kernel(**inputs) -> np.ndarray:
    out, _ = run(inputs, trace=False)
    return out


# revision 20
# speedup vs baseline: 1.6357x; 1.6357x over previous
"""Masked per-sample MSE loss (duration-predictor loss) on 8 Trainium2 cores.

Math (per the reference):
    mask[i, j]  = j < token_lengths[i]
    diff        = where(mask, pred - log(alignment), 0.0)
    out         = mean_i( sum_j diff[i,j]^2 / token_lengths[i] )

Sharding: data parallel over the batch dim, with length-sorted row
assignment. Rows are sorted by token_length; rank r goes to row-tile
t = r // 1024, core c = r % 8, partition p = (r % 1024) // 8. Every core's
row-tile t then spans the same global length range, so a single SPMD module
(shapes fixed at build time from the global per-tile max lengths W[t]) fits
all cores, and each tile only needs its first W[t] columns DMA'd — roughly
62% of the full input traffic for uniform lengths. Columns are processed in
"bands" [W[b-1], W[b]) covering tiles b..3 in one DMA / one fused op each;
only the diagonal tile of a band can contain the mask boundary (sorted
order guarantees every earlier tile is fully valid there), so the masking
op runs only on diagonal slices.

Device pipeline per unit (column range x active tiles):
    ACT : la = Ln(align)                      (folded over active tiles)
    DVE : d  = pred - la                      (folded over active tiles)
    DVE : dm = (iota < len) * d               (diagonal tile slice only)
    ACT : sq = Square(d or dm), accum_out -> rs[:, k]   (per tile)
The per-row divide by length and the global mean run on the host in f64.

Written in raw Bass (explicit semaphores): the walrus build in this
environment rejects compute instructions carrying more than one sync-wait,
so waits are issued as standalone wait_ge instructions.
"""

from contextlib import ExitStack

import numpy as np

import concourse.bass as bass
from concourse import mybir
from concourse.bass_utils import run_bass_kernel_spmd

B, T = 4096, 2048
N_CORES = 8
RPC = B // N_CORES    # rows per core = 512
P = 128               # SBUF partitions
N_TILES = RPC // P    # row-tiles per core = 4
GROUP = P * N_CORES   # sorted ranks per row-tile = 1024

_CACHE: dict = {}

F32 = mybir.dt.float32


def _tail_chunks(width):
    """Split the final band into shrinking chunks so the last dependency
    chain runs on a small piece."""
    if width <= 128:
        return [width]
    chunks = []
    rem = width
    while rem > 768:
        take = min(1024, rem - 512)
        chunks.append(take)
        rem -= take
    # rem in (96, 768]: halve down to ~64
    while rem > 96:
        take = max(64, rem // 2)
        chunks.append(take)
        rem -= take
    chunks.append(rem)
    return chunks


MAXDW = 4096  # cap on tiles*width per folded work unit (d_sb slot size)


def _build_plan(W):
    """units: list of dicts with col offset/width, active tiles, diag tile.
    sq list: (unit_idx, tile, rs_col)."""
    units = []
    prev = 0
    for b in range(N_TILES):
        hi = W[b]
        if hi <= prev:
            continue
        n = N_TILES - b
        if b == N_TILES - 1:
            widths = _tail_chunks(hi - prev)
        else:
            widths, rem = [], hi - prev
            while rem > 0:
                take = min(rem, MAXDW // n)
                widths.append(take)
                rem -= take
        o = prev
        for w in widths:
            units.append({
                "o": o, "w": w,
                "tiles": list(range(b, N_TILES)),
                "diag": b,
            })
            o += w
        prev = hi
    # rs column per (unit, tile)
    col = 0
    for u in units:
        u["rs"] = {}
        for t in u["tiles"]:
            u["rs"][t] = col
            col += 1
    return units, col


def _build_module(W) -> tuple[bass.Bass, list, int]:
    units, n_rs = _build_plan(W)
    nu = len(units)

    nc = bass.Bass("TRN2")

    pred_d = nc.dram_tensor("pred", [RPC, T], F32, kind="ExternalInput")
    align_d = nc.dram_tensor("align", [RPC, T], F32, kind="ExternalInput")
    lens_d = nc.dram_tensor("lens", [P, N_TILES], F32, kind="ExternalInput")
    out_d = nc.dram_tensor("rowsums", [P, n_rs], F32, kind="ExternalOutput")

    with ExitStack() as ctx:
        pred_sb = ctx.enter_context(nc.sbuf_tensor("pred_sb", [P, N_TILES, T], F32))
        align_sb = ctx.enter_context(nc.sbuf_tensor("align_sb", [P, N_TILES, T], F32))
        # Ln runs in place: la overwrites align
        # d per unit: [tiles x width] flattened into one row of d_sb slots
        d_sb = ctx.enter_context(nc.sbuf_tensor("d_sb", [P, 2, MAXDW], F32))
        dm_sb = ctx.enter_context(nc.sbuf_tensor("dm_sb", [P, 4, 2048], F32))
        sq_sb = ctx.enter_context(nc.sbuf_tensor("sq_sb", [P, 2, 2048], F32))
        iota_i = ctx.enter_context(nc.sbuf_tensor("iota_i", [P, T], mybir.dt.int32))
        iota_f = ctx.enter_context(nc.sbuf_tensor("iota_f", [P, T], F32))
        lens_sb = ctx.enter_context(nc.sbuf_tensor("lens_sb", [P, N_TILES], F32))
        rs_sb = ctx.enter_context(nc.sbuf_tensor("rs_sb", [P, n_rs], F32))
        s_pred = [ctx.enter_context(nc.semaphore(f"s_pred{u}")) for u in range(nu)]
        s_align = [ctx.enter_context(nc.semaphore(f"s_align{u}")) for u in range(nu)]
        s_lens = ctx.enter_context(nc.semaphore("s_lens"))
        s_out = ctx.enter_context(nc.semaphore("s_out"))
        s_iota = ctx.enter_context(nc.semaphore("s_iota"))
        s_iotaf = ctx.enter_context(nc.semaphore("s_iotaf"))
        s_la = [ctx.enter_context(nc.semaphore(f"s_la{u}")) for u in range(nu)]
        s_d = ctx.enter_context(nc.semaphore("s_d"))
        s_dm = ctx.enter_context(nc.semaphore("s_dm"))
        s_sqa = ctx.enter_context(nc.semaphore("s_sqa"))
        s_sqv = ctx.enter_context(nc.semaphore("s_sqv"))
        block = ctx.enter_context(nc.Block())

        def dram_band(dram, u):
            t0 = u["tiles"][0]
            n = len(u["tiles"])
            cols = slice(u["o"], u["o"] + u["w"])
            ap = dram[t0 * P:(t0 + n) * P, cols]
            return ap.rearrange("(n p) w -> p n w", p=P)

        def sbuf_band(sb, u):
            t0 = u["tiles"][0]
            n = len(u["tiles"])
            return sb[:, t0:t0 + n, u["o"]:u["o"] + u["w"]]

        nu_diag = nu  # one diagonal square per unit

        # aligns of the last few (tiny) units are DMA'd early so their Ln
        # is long done when their pred arrives — the final dependency chain
        # after the last DMA byte is then just d -> mask -> Square.
        hoist = set(range(max(1, nu - 3), nu))
        ln_order = [0] + sorted(hoist) + [u for u in range(1, nu) if u not in hoist]

        @block.sync
        def _(sync):
            def dma_a(ui):
                u = units[ui]
                sync.dma_start(sbuf_band(align_sb, u), dram_band(align_d, u)
                               ).then_inc(s_align[ui], 16)

            def dma_p(ui):
                u = units[ui]
                sync.dma_start(sbuf_band(pred_sb, u), dram_band(pred_d, u)
                               ).then_inc(s_pred[ui], 16)

            for ui in ln_order[:1 + len(hoist)]:
                dma_a(ui)
            dma_p(0)
            for ui in range(1, nu):
                if ui not in hoist:
                    dma_a(ui)
                dma_p(ui)
            sync.wait_ge(s_sqa, nu_diag)
            sync.wait_ge(s_sqv, n_rs - nu_diag)
            sync.dma_start(out_d[:, :], rs_sb[:, :]).then_inc(s_out, 16)
            sync.wait_ge(s_out, 16)

        @block.gpsimd
        def _(gpsimd):
            # lens via the SWDGE queue: keeps the HWDGE ring for bulk traffic
            gpsimd.dma_start(lens_sb[:, :], lens_d[:, :]).then_inc(s_lens, 16)
            gpsimd.iota(
                iota_i[:, :], pattern=[[1, T]], base=0, channel_multiplier=0
            ).then_inc(s_iota, 1)

        @block.vector
        def _(vector):
            vector.wait_ge(s_iota, 1)
            vector.tensor_copy(iota_f[:, :], iota_i[:, :]).then_inc(s_iotaf, 1)
            vector.wait_ge(s_lens, 16)
            vector.wait_ge(s_iotaf, 1)  # same-engine RAW: iota_f visible
            n_d = 0
            for ui, u in enumerate(units):
                n = len(u["tiles"])
                w = u["w"]
                vector.wait_ge(s_pred[ui], 16)
                vector.wait_ge(s_la[ui], 1)
                dslot = d_sb[:, ui % 2, :n * w].rearrange(
                    "p (n w) -> p n w", n=n)
                vector.tensor_sub(
                    dslot, sbuf_band(pred_sb, u), sbuf_band(align_sb, u)
                ).then_inc(s_d, 1)
                n_d += 1
                dt = u["diag"]
                di = u["tiles"].index(dt)
                if ui >= 4:
                    # dm slot reuse: diagonal Square of unit ui-4 done
                    vector.wait_ge(s_sqa, ui - 3)
                vector.wait_ge(s_d, n_d)  # same-engine RAW: d visible
                vector.scalar_tensor_tensor(
                    out=dm_sb[:, ui % 4, :w],
                    in0=iota_f[:, u["o"]:u["o"] + w],
                    scalar=lens_sb[:, dt:dt + 1],
                    in1=dslot[:, di, :],
                    op0=mybir.AluOpType.is_lt,
                    op1=mybir.AluOpType.mult,
                ).then_inc(s_dm, 1)
                # off-diagonal squares on DVE: in-place d*d with row-sum
                for t in u["tiles"]:
                    if t == dt:
                        continue
                    ti = u["tiles"].index(t)
                    rcol = u["rs"][t]
                    vector.scalar_tensor_tensor(
                        out=dslot[:, ti, :],
                        in0=dslot[:, ti, :],
                        scalar=1.0,
                        in1=dslot[:, ti, :],
                        op0=mybir.AluOpType.mult,
                        op1=mybir.AluOpType.mult,
                        accum_out=rs_sb[:, rcol:rcol + 1],
                    ).then_inc(s_sqv, 1)

        @block.scalar
        def _(scalar):
            n_sq = 0

            def ln(ui):
                u = units[ui]
                scalar.wait_ge(s_align[ui], 16)
                scalar.activation(
                    sbuf_band(align_sb, u), sbuf_band(align_sb, u),
                    mybir.ActivationFunctionType.Ln,
                ).then_inc(s_la[ui], 1)

            def sq_diag(ui):
                nonlocal n_sq
                u = units[ui]
                w = u["w"]
                dt = u["diag"]
                rcol = u["rs"][dt]
                scalar.wait_ge(s_dm, ui + 1)
                if n_sq >= 2:
                    # same-engine WAW on alternating sq_sb scratch
                    scalar.wait_ge(s_sqa, n_sq - 1)
                scalar.activation(
                    sq_sb[:, n_sq % 2, :w], dm_sb[:, ui % 4, :w],
                    mybir.ActivationFunctionType.Square,
                    accum_out=rs_sb[:, rcol:rcol + 1],
                ).then_inc(s_sqa, 1)
                n_sq += 1

            # Lns in align-arrival order (tail Lns hoisted), Squares
            # interleaved as their dm becomes available
            lns = list(ln_order)
            n_emitted = 0
            for k in range(1 + len(hoist) + 1):
                if lns:
                    ln(lns.pop(0))
            while lns:
                sq_diag(n_emitted)
                n_emitted += 1
                ln(lns.pop(0))
            while n_emitted < nu:
                sq_diag(n_emitted)
                n_emitted += 1

    return nc, units, n_rs


def _get_module(W):
    key = tuple(W)
    if key not in _CACHE:
        _CACHE[key] = _build_module(W)
    return _CACHE[key]


def _plan_sharding(lens):
    """Sorted, rank-interleaved sharding. Returns (rows[c][t*P+p] global row
    ids per core, W per-tile max lengths)."""
    order = np.argsort(lens, kind="stable")
    W = []
    for t in range(N_TILES):
        grp = lens[order[t * GROUP:(t + 1) * GROUP]]
        W.append(int(grp.max()))
    rows = []
    for c in range(N_CORES):
        ids = np.empty(RPC, dtype=np.int64)
        for t in range(N_TILES):
            # partition p gets sorted rank t*GROUP + p*N_CORES + c
            ids[t * P:(t + 1) * P] = order[
                t * GROUP + c + N_CORES * np.arange(P)]
        rows.append(ids)
    return rows, W


def run(inputs, trace: bool = False):
    """Returns (output, BassKernelResults). trace=True also profiles core 0."""
    pred = np.asarray(inputs["pred"], dtype=np.float32)
    align = np.asarray(inputs["alignment"], dtype=np.float32)
    lens = np.asarray(inputs["token_lengths"])

    rows, W = _plan_sharding(lens)
    nc, units, n_rs = _get_module(W)

    in_maps = []
    for c in range(N_CORES):
        ids = rows[c]
        lens_c = lens[ids].astype(np.float32)
        in_maps.append({
            "pred": np.ascontiguousarray(pred[ids]),
            "align": np.ascontiguousarray(align[ids]),
            "lens": np.ascontiguousarray(lens_c.reshape(N_TILES, P).T),
        })

    res = run_bass_kernel_spmd(nc, in_maps, core_ids=list(range(N_CORES)), trace=trace)

    total = 0.0
    for c in range(N_CORES):
        rs = np.asarray(res.results[c]["rowsums"], dtype=np.float64)  # [P, n_rs]
        rows_sum = np.zeros((P, N_TILES))
        for u in units:
            for t, rcol in u["rs"].items():
                rows_sum[:, t] += rs[:, rcol]
        per_row = rows_sum.T.reshape(RPC)
        lc = lens[rows[c]].astype(np.float64)
        total += np.sum(per_row / lc)
    return np.array(total / B, dtype=np.float32), res


def kernel(**inputs) -> np.ndarray:
    out, _ = run(inputs, trace=False)
    return out
